# revision 42
# baseline (speedup 1.0000x reference)
"""Trainium2 Bass kernel for LocallyConnected1D (filters=1, k=1, no bias):

    out[b, s, 0] = sum_c x[b, s, c] * W[s, c]

x: (256, 8192, 64) f32, W: (8192, 64) f32, out: (256, 8192, 1) f32.

Strategy: data-parallel over batch across the 8 NeuronCores (32
batches/core, W replicated, no collectives).  Memory-bound: 64 MiB of x
per core must stream from HBM; the kernel sustains ~395 GB/s read-side
by striping every tile across two DMA queues.

Per core, a tile holds 2 batches in s-major layout: partition p owns
s in [64p, 64p+64) and the free dim is (batch, s_local, c) = 8192.
With both batch halves sharing the partition->s map, W is a single
[128, 4096] bf16 tile (no replication): one 2 MiB cast-load at the
front of the gpsimd queue, which is what lets the first mul start ~35us
instead of ~45us.  Each tile streams as its two batches in parallel:
  - batch 0 on the sync HWDGE queue as fp32, cast to bf16 on the
    otherwise-idle ACT engine,
  - batch 1 on the SWDGE queue (nc.gpsimd) with the fp32->bf16 cast
    done inside the DMA.
Compute runs in bf16 on DVE: in-place tensor_mul per batch slice
(2x_1p mode), then ONE group-of-64 reduction over the whole tile as a
pairwise-add tree (bf16 TT adds also run 2x; tensor_reduce is stuck at
1x), last level into fp32.  Out tiles store on the scalar queue two
tiles late; the last few store via the gpsimd queue after the loop so
the ACT cast stream never blocks on late compute.  The final tile's
fp32 batch loads and casts in two chunks so the tail stays short.
"""

import sys
from contextlib import ExitStack

import numpy as np

for _p in ("/opt/trn_rl_repo", "/root/.axon_site/_ro/trn_rl_repo"):
    if _p not in sys.path:
        sys.path.insert(0, _p)

import concourse.bacc as bacc
import concourse.mybir as mybir
import concourse.tile as tile
from concourse.bass import broadcast_tensor_aps
from concourse.bass_utils import run_bass_kernel_spmd

B, S, C = 256, 8192, 64
NCORES = 8
BPC = B // NCORES          # 32 batches per core
BPT = 2                    # batches per tile
NT = BPC // BPT            # 16 tiles per core
P = 128
FREE = BPT * S * C // P    # 8192 elems per partition line
HF = FREE // 2             # 4096: one batch's slice
JP = BPT * S // P          # 128 outputs per partition line
PREFETCH = 3
ACOL = 3584                # batch-0 cols on the sync queue (fp32)
BCOL = 3584                # cols on the scalar queue (fp32): rest of
                           # batch 0 plus the head of batch 1
GCOL = FREE - ACOL - BCOL  # 1024: SWDGE cast-DMA slice (tail of batch 1)

_cache = {}

BF16 = mybir.dt.bfloat16
F32 = mybir.dt.float32


def _build():
    nc = bacc.Bacc("TRN2", debug=False, target_bir_lowering=False)
    x = nc.dram_tensor("x", [BPC * S * C], F32, kind="ExternalInput").ap()
    w = nc.dram_tensor("w", [S * C], F32, kind="ExternalInput").ap()
    out = nc.dram_tensor("out", [BPC * S], F32, kind="ExternalOutput").ap()

    # s-major pair layout: [tile, batch-in-pair, partition, f=(s_local c)]
    x_v = x.rearrange("(i b p f) -> i b p f", i=NT, b=BPT, p=P)  # [16,2,128,4096]
    w_v = w.rearrange("(p f) -> p f", p=P)                       # [128, 4096]
    o_v = out.rearrange("(i b p j) -> i p b j", i=NT, b=BPT, p=P)  # [16,128,2,64]

    with tile.TileContext(nc) as tc, ExitStack() as ctx:
        xp = ctx.enter_context(tc.tile_pool(name="xp", bufs=4))
        xfap = ctx.enter_context(tc.tile_pool(name="xfap", bufs=3))
        xfbp = ctx.enter_context(tc.tile_pool(name="xfbp", bufs=4))
        wp = ctx.enter_context(tc.tile_pool(name="wp", bufs=1))
        t1p = ctx.enter_context(tc.tile_pool(name="t1p", bufs=2))
        s2p = ctx.enter_context(tc.tile_pool(name="s2p", bufs=2))
        op = ctx.enter_context(tc.tile_pool(name="op", bufs=4))

        # W: ONE SWDGE cast-load (no replication needed in this layout),
        # at the front of the (lightly loaded) gpsimd queue.
        wt = wp.tile([P, HF], BF16)
        nc.gpsimd.dma_start(wt[:], w_v[:, :])

        xts = []
        xfs = {}

        def issue_loads(i):
            xt = xp.tile([P, FREE], BF16)
            xfa = xfap.tile([P, ACOL], F32)
            xfb = xfbp.tile([P, BCOL], F32)
            nc.sync.dma_start(xfa[:], x_v[i, 0][:, 0:ACOL])
            nc.scalar.dma_start(xfb[:, 0 : HF - ACOL], x_v[i, 0][:, ACOL:HF])
            nc.scalar.dma_start(
                xfb[:, HF - ACOL : BCOL], x_v[i, 1][:, 0 : ACOL + BCOL - HF]
            )
            nc.gpsimd.dma_start(
                xt[:, ACOL + BCOL : FREE], x_v[i, 1][:, ACOL + BCOL - HF : HF]
            )
            xts.append(xt)
            xfs[i] = (xfa, xfb)

        for i in range(min(PREFETCH, NT)):
            issue_loads(i)

        ots = []
        for i in range(NT):
            if i + PREFETCH < NT:
                issue_loads(i + PREFETCH)
            xt = xts[i]
            xfa, xfb = xfs.pop(i)
            nc.scalar.copy(xt[:, 0:ACOL], xfa[:])
            nc.scalar.copy(xt[:, ACOL : ACOL + BCOL], xfb[:])

            # One mul per batch slice; both read the same W tile.  The
            # tile_wait_until pins each tile's muls at a 7us/tile cadence
            # in the scheduler's model: real arrivals (~10.6us/tile) are
            # always later so the wait never binds at runtime, but the
            # static schedule can no longer hoist a late tile's muls above
            # an earlier tile's ready tree (in-order DVE head-of-line
            # blocking cost ~15us at the tail otherwise).
            with tc.tile_wait_until(0.007 * i):
                # One fused mul for both batch slices: W broadcast along
                # the batch sub-dim via a stride-0 AP (innermost dim stays
                # contiguous bf16, so the 2x_1p mode is preserved).
                xb = xt[:].rearrange("p (b f) -> p b f", b=BPT)
                wb = wt[:].rearrange("p (o f) -> p o f", o=1)
                xb2, wb2 = broadcast_tensor_aps(xb, wb)
                nc.vector.tensor_mul(xb2, xb2, wb2)

            # Group-of-64 reduction: bf16 pairwise-add tree on DVE.
            x3 = xt[:].rearrange("p (j c) -> p j c", c=C)          # [p,128,64]
            t1 = t1p.tile([P, JP * 32], BF16)
            t1v = t1[:].rearrange("p (j c) -> p j c", c=32)
            nc.vector.tensor_add(t1v, x3[:, :, 0:32], x3[:, :, 32:64])
            s2 = s2p.tile([P, JP * 30], BF16)
            l2 = s2[:, 0 : JP * 16].rearrange("p (j c) -> p j c", c=16)
            nc.vector.tensor_add(l2, t1v[:, :, 0:16], t1v[:, :, 16:32])
            l3 = s2[:, JP * 16 : JP * 24].rearrange("p (j c) -> p j c", c=8)
            nc.vector.tensor_add(l3, l2[:, :, 0:8], l2[:, :, 8:16])
            l4 = s2[:, JP * 24 : JP * 28].rearrange("p (j c) -> p j c", c=4)
            nc.vector.tensor_add(l4, l3[:, :, 0:4], l3[:, :, 4:8])
            l5 = s2[:, JP * 28 : JP * 30].rearrange("p (j c) -> p j c", c=2)
            nc.vector.tensor_add(l5, l4[:, :, 0:2], l4[:, :, 2:4])
            ot = op.tile([P, JP], F32)
            o3 = ot[:].rearrange("p (j c) -> p j c", c=1)
            nc.vector.tensor_add(o3, l5[:, :, 0:1], l5[:, :, 1:2])
            ots.append(ot)
            # All stores via gpsimd (nearly idle queue), two tiles late
            # so the tree-wait never stalls the x-slice descriptor gen.
            if i >= 2:
                nc.gpsimd.dma_start(
                    o_v[i - 2], ots[i - 2][:].rearrange("p (b j) -> p b j", b=BPT)
                )
        for i in range(NT - 2, NT):
            nc.gpsimd.dma_start(o_v[i], ots[i][:].rearrange("p (b j) -> p b j", b=BPT))

    nc.compile()
    return nc


def _get_nc():
    if "nc" not in _cache:
        _cache["nc"] = _build()
    return _cache["nc"]


def run_sharded(x, W, **spmd_kwargs):
    """Shard, run on 8 cores, gather. Returns (out[B, S], BassKernelResults)."""
    nc = _get_nc()
    xf = np.ascontiguousarray(x, dtype=np.float32).reshape(NCORES, BPC * S * C)
    wf = np.ascontiguousarray(W, dtype=np.float32).reshape(S * C)
    in_maps = [{"x": xf[i], "w": wf} for i in range(NCORES)]
    r = run_bass_kernel_spmd(nc, in_maps, list(range(NCORES)), **spmd_kwargs)
    out = np.concatenate(
        [np.asarray(r.results[i]["out"]).reshape(BPC, S) for i in range(NCORES)],
        axis=0,
    )
    return out, r


def kernel(x, W):
    out, _ = run_sharded(x, W)
    return out[..., None].astype(np.float32)


# revision 43
# speedup vs baseline: 1.1598x; 1.1598x over previous
"""Trainium2 Bass kernel for LocallyConnected1D (filters=1, k=1, no bias):

    out[b, s, 0] = sum_c x[b, s, c] * W[s, c]

x: (256, 8192, 64) f32, W: (8192, 64) f32, out: (256, 8192, 1) f32.

Strategy: data-parallel over batch across the 8 NeuronCores (32
batches/core, W replicated, no collectives).  Memory-bound: 64 MiB of x
per core must stream from HBM; the kernel sustains ~395 GB/s read-side
by striping every tile across two DMA queues.

Per core, a tile holds 2 batches in s-major layout: partition p owns
s in [64p, 64p+64) and the free dim is (batch, s_local, c) = 8192.
With both batch halves sharing the partition->s map, W is a single
[128, 4096] bf16 tile (no replication): one 2 MiB cast-load at the
front of the gpsimd queue, which is what lets the first mul start ~35us
instead of ~45us.  Each tile streams as its two batches in parallel:
  - batch 0 on the sync HWDGE queue as fp32, cast to bf16 on the
    otherwise-idle ACT engine,
  - batch 1 on the SWDGE queue (nc.gpsimd) with the fp32->bf16 cast
    done inside the DMA.
Compute runs in bf16 on DVE: in-place tensor_mul per batch slice
(2x_1p mode), then ONE group-of-64 reduction over the whole tile as a
pairwise-add tree (bf16 TT adds also run 2x; tensor_reduce is stuck at
1x), last level into fp32.  Out tiles store on the scalar queue two
tiles late; the last few store via the gpsimd queue after the loop so
the ACT cast stream never blocks on late compute.  The final tile's
fp32 batch loads and casts in two chunks so the tail stays short.
"""

import sys
from contextlib import ExitStack

import numpy as np

for _p in ("/opt/trn_rl_repo", "/root/.axon_site/_ro/trn_rl_repo"):
    if _p not in sys.path:
        sys.path.insert(0, _p)

import concourse.bacc as bacc
import concourse.mybir as mybir
import concourse.tile as tile
from concourse.bass import broadcast_tensor_aps
from concourse.bass_utils import run_bass_kernel_spmd

B, S, C = 256, 8192, 64
NCORES = 8
BPC = B // NCORES          # 32 batches per core
BPT = 2                    # batches per tile
NT = BPC // BPT            # 16 tiles per core
P = 128
FREE = BPT * S * C // P    # 8192 elems per partition line
HF = FREE // 2             # 4096: one batch's slice
JP = BPT * S // P          # 128 outputs per partition line
PREFETCH = 3
ACOL = 3584                # batch-0 cols on the sync queue (fp32)
BCOL = 3584                # cols on the scalar queue (fp32): rest of
                           # batch 0 plus the head of batch 1
GCOL = FREE - ACOL - BCOL  # 1024: SWDGE cast-DMA slice (tail of batch 1)

_cache = {}

BF16 = mybir.dt.bfloat16
F32 = mybir.dt.float32


def _build():
    nc = bacc.Bacc("TRN2", debug=False, target_bir_lowering=False)
    x = nc.dram_tensor("x", [BPC * S * C], F32, kind="ExternalInput").ap()
    w = nc.dram_tensor("w", [S * C], F32, kind="ExternalInput").ap()
    out = nc.dram_tensor("out", [BPC * S], F32, kind="ExternalOutput").ap()

    # s-major pair layout: [tile, batch-in-pair, partition, f=(s_local c)]
    x_v = x.rearrange("(i b p f) -> i b p f", i=NT, b=BPT, p=P)  # [16,2,128,4096]
    w_v = w.rearrange("(p f) -> p f", p=P)                       # [128, 4096]
    o_v = out.rearrange("(i b p j) -> i p b j", i=NT, b=BPT, p=P)  # [16,128,2,64]

    with tile.TileContext(nc) as tc, ExitStack() as ctx:
        xp = ctx.enter_context(tc.tile_pool(name="xp", bufs=4))
        xfap = ctx.enter_context(tc.tile_pool(name="xfap", bufs=3))
        xfbp = ctx.enter_context(tc.tile_pool(name="xfbp", bufs=3))
        wp = ctx.enter_context(tc.tile_pool(name="wp", bufs=1))
        t1p = ctx.enter_context(tc.tile_pool(name="t1p", bufs=2))
        s2p = ctx.enter_context(tc.tile_pool(name="s2p", bufs=2))
        op = ctx.enter_context(tc.tile_pool(name="op", bufs=4))

        # W: ONE SWDGE cast-load (no replication needed in this layout),
        # at the front of the (lightly loaded) gpsimd queue.
        wt = wp.tile([P, HF], BF16)
        nc.gpsimd.dma_start(wt[:], w_v[:, :])

        xts = []
        xfs = {}

        def issue_loads(i):
            xt = xp.tile([P, FREE], BF16)
            xfa = xfap.tile([P, ACOL], F32)
            xfb = xfbp.tile([P, BCOL], F32)
            nc.sync.dma_start(xfa[:], x_v[i, 0][:, 0:ACOL])
            nc.scalar.dma_start(xfb[:, 0 : HF - ACOL], x_v[i, 0][:, ACOL:HF])
            nc.scalar.dma_start(
                xfb[:, HF - ACOL : BCOL], x_v[i, 1][:, 0 : ACOL + BCOL - HF]
            )
            nc.gpsimd.dma_start(
                xt[:, ACOL + BCOL : FREE], x_v[i, 1][:, ACOL + BCOL - HF : HF]
            )
            xts.append(xt)
            xfs[i] = (xfa, xfb)

        for i in range(min(PREFETCH, NT)):
            issue_loads(i)

        ots = []
        for i in range(NT):
            if i + PREFETCH < NT:
                issue_loads(i + PREFETCH)
            xt = xts[i]
            xfa, xfb = xfs.pop(i)
            nc.scalar.copy(xt[:, 0:ACOL], xfa[:])
            nc.scalar.copy(xt[:, ACOL : ACOL + BCOL], xfb[:])

            # One mul per batch slice; both read the same W tile.  The
            # tile_wait_until pins each tile's muls at a 7us/tile cadence
            # in the scheduler's model: real arrivals (~10.6us/tile) are
            # always later so the wait never binds at runtime, but the
            # static schedule can no longer hoist a late tile's muls above
            # an earlier tile's ready tree (in-order DVE head-of-line
            # blocking cost ~15us at the tail otherwise).
            with tc.tile_wait_until(0.007 * i):
                # One fused mul for both batch slices: W broadcast along
                # the batch sub-dim via a stride-0 AP (innermost dim stays
                # contiguous bf16, so the 2x_1p mode is preserved).
                xb = xt[:].rearrange("p (b f) -> p b f", b=BPT)
                wb = wt[:].rearrange("p (o f) -> p o f", o=1)
                xb2, wb2 = broadcast_tensor_aps(xb, wb)
                nc.vector.tensor_mul(xb2, xb2, wb2)

            # Group-of-64 reduction: bf16 pairwise-add tree on DVE.
            x3 = xt[:].rearrange("p (j c) -> p j c", c=C)          # [p,128,64]
            t1 = t1p.tile([P, JP * 32], BF16)
            t1v = t1[:].rearrange("p (j c) -> p j c", c=32)
            nc.vector.tensor_add(t1v, x3[:, :, 0:32], x3[:, :, 32:64])
            s2 = s2p.tile([P, JP * 30], BF16)
            l2 = s2[:, 0 : JP * 16].rearrange("p (j c) -> p j c", c=16)
            nc.vector.tensor_add(l2, t1v[:, :, 0:16], t1v[:, :, 16:32])
            l3 = s2[:, JP * 16 : JP * 24].rearrange("p (j c) -> p j c", c=8)
            nc.vector.tensor_add(l3, l2[:, :, 0:8], l2[:, :, 8:16])
            l4 = s2[:, JP * 24 : JP * 28].rearrange("p (j c) -> p j c", c=4)
            nc.vector.tensor_add(l4, l3[:, :, 0:4], l3[:, :, 4:8])
            l5 = s2[:, JP * 28 : JP * 30].rearrange("p (j c) -> p j c", c=2)
            nc.vector.tensor_add(l5, l4[:, :, 0:2], l4[:, :, 2:4])
            ot = op.tile([P, JP], F32)
            o3 = ot[:].rearrange("p (j c) -> p j c", c=1)
            nc.vector.tensor_add(o3, l5[:, :, 0:1], l5[:, :, 1:2])
            ots.append(ot)
            # All stores via gpsimd (nearly idle queue), two tiles late
            # so the tree-wait never stalls the x-slice descriptor gen.
            if i >= 2:
                nc.gpsimd.dma_start(
                    o_v[i - 2], ots[i - 2][:].rearrange("p (b j) -> p b j", b=BPT)
                )
        for i in range(NT - 2, NT):
            nc.gpsimd.dma_start(o_v[i], ots[i][:].rearrange("p (b j) -> p b j", b=BPT))

    nc.compile()
    return nc


def _get_nc():
    if "nc" not in _cache:
        _cache["nc"] = _build()
    return _cache["nc"]


def run_sharded(x, W, **spmd_kwargs):
    """Shard, run on 8 cores, gather. Returns (out[B, S], BassKernelResults)."""
    nc = _get_nc()
    xf = np.ascontiguousarray(x, dtype=np.float32).reshape(NCORES, BPC * S * C)
    wf = np.ascontiguousarray(W, dtype=np.float32).reshape(S * C)
    in_maps = [{"x": xf[i], "w": wf} for i in range(NCORES)]
    r = run_bass_kernel_spmd(nc, in_maps, list(range(NCORES)), **spmd_kwargs)
    out = np.concatenate(
        [np.asarray(r.results[i]["out"]).reshape(BPC, S) for i in range(NCORES)],
        axis=0,
    )
    return out, r


def kernel(x, W):
    out, _ = run_sharded(x, W)
    return out[..., None].astype(np.float32)
